# revision 54
# baseline (speedup 1.0000x reference)
"""CBAM (channel attention + non-local spatial attention) Trainium2 kernel.

Full-input contract: kernel(**inputs) takes the complete tensors as produced by
setup_inputs() and returns the full [8, 256, 64, 64] output. Internally the
batch dimension (8) is sharded 1:1 across 8 NeuronCores; every core holds the
full (tiny) weights.

Per-core math (batch b, x viewed as [C=256, HW=4096]):
  gate  = sigmoid(W2 relu(W1 avg) + W2 relu(W1 max))          (channel attn)
  xg    = x * gate
  q,k   = Wq/Wk @ xg + b                                      [16, HW]
  vT    = gamma * (xg^T @ Wv^T + v_b)                         [HW, 256]
  S^T   = k^T q                                               [j, i] layout
  P     = exp(S^T)            (scores are O(6); no max-subtraction needed)
  out2  = P^T @ [vT | 1]      -> [i, 256 cols of gamma*V@attn | denom]
  y     = transpose(out2[:, :256] / denom) + xg

All attention-side matmul operands are bf16: the PE streams bf16 at 1
cycle/row (vs 4 for plain fp32) and 128-col bf16 weights get the
compiler's fast-weight-load (2 elems/read), which hides LDWEIGHTS under
the previous matmul's streaming. The channel-attention gate path and the
residual xg stay fp32, so bf16 error only enters through the
gamma~0.05-scaled attention term (~1e-4 end-to-end).
"""

import numpy as np
from contextlib import ExitStack

import ml_dtypes

import concourse.bass as bass
import concourse.tile as tile
from concourse import bacc, mybir
from concourse.bass_utils import run_bass_kernel_spmd
from concourse.masks import make_identity

B, C, H, W = 8, 256, 64, 64
HW = H * W            # 4096
R = 16                # C // reduction
P = 128               # SBUF partitions
CT = C // P           # 2 channel tiles
NJ = HW // P          # 32 key tiles
SI = 512              # query super-block
NI = HW // SI         # 8 super-blocks
F32 = mybir.dt.float32
BF16 = mybir.dt.bfloat16

AF = mybir.ActivationFunctionType


def _build_nc():
    nc = bacc.Bacc("TRN2", target_bir_lowering=False)
    x_d = nc.declare_dram_parameter("x", [C, HW], F32, isOutput=False)
    # all fp32 weights packed in one DRAM blob, all bf16 weights in a
    # second: 2 weight DMAs instead of 6 (fewer DMAs -> no semaphore-pool
    # collision with the x chunk stream)
    # fp32 blob columns: w1t [0:32], vbc [32:34], qkb [34:35] (rows 0-63),
    #                    w2t [35:291] (rows 0-15)
    wf_d = nc.declare_dram_parameter("wf", [P, 291], F32, isOutput=False)
    # bf16 blob columns: qkwt [0:128], vwt [128:640]
    wb_d = nc.declare_dram_parameter("wb", [P, 640], BF16, isOutput=False)
    g_d = nc.declare_dram_parameter("gamma", [1, 1], F32, isOutput=False)
    y_d = nc.declare_dram_parameter("y", [C, HW], F32, isOutput=True)

    with ExitStack() as ctx:
        tc = ctx.enter_context(tile.TileContext(nc))
        const = ctx.enter_context(tc.tile_pool(name="const", bufs=1))
        xp = ctx.enter_context(tc.tile_pool(name="xp", bufs=1))
        xgp = ctx.enter_context(tc.tile_pool(name="xgp", bufs=1))
        qkp = ctx.enter_context(tc.tile_pool(name="qkp", bufs=1))
        vtp = ctx.enter_context(tc.tile_pool(name="vtp", bufs=1))
        # pt double-buffered: exp(si+1) overwrites pt while out2(si) still
        # streams the previous generation -> ACT and PE stay overlapped
        pp = ctx.enter_context(tc.tile_pool(name="pp", bufs=2))
        NQ = NJ // 4          # 8 score quads per si block
        small = ctx.enter_context(tc.tile_pool(name="small", bufs=4))
        outp = ctx.enter_context(tc.tile_pool(name="outp", bufs=4))
        # PSUM: 4 (score quad, single-buffered) + 2 (out2/pj) + 2
        # (transpose/junk) = 8 banks
        ps_s = ctx.enter_context(tc.tile_pool(name="ps_s", bufs=1, space="PSUM"))
        ps_o = ctx.enter_context(tc.tile_pool(name="ps_o", bufs=2, space="PSUM"))
        ps_t = ctx.enter_context(tc.tile_pool(name="ps_t", bufs=2, space="PSUM"))

        # ---------------- loads ----------------
        # A dma_start occupies its issuing engine for the whole transfer,
        # so each data stream gets a dedicated engine: x ct0 on sync, x ct1
        # on gpsimd (idle until the y stores much later), weights on scalar
        # (tiny, emitted ahead of the scalar-engine x16 casts).
        NCH = 4
        x_sb = []
        for ct in range(CT):
            xt = xp.tile([P, HW], F32, name=f"x{ct}", tag=f"x{ct}")
            x_sb.append(xt)
        for ch in range(NCH):
            for ct in range(CT):
                eng = nc.sync if ct == 0 else nc.gpsimd
                eng.dma_start(
                    out=x_sb[ct][:, ch * (HW // NCH):(ch + 1) * (HW // NCH)],
                    in_=x_d[ct * P:(ct + 1) * P,
                            ch * (HW // NCH):(ch + 1) * (HW // NCH)])

        wf = const.tile([P, 291], F32, name="wf")
        nc.scalar.dma_start(out=wf, in_=wf_d[:, :])
        wb = const.tile([P, 640], BF16, name="wb")
        nc.scalar.dma_start(out=wb, in_=wb_d[:, :])
        w1t = wf[:, 0:CT * R]
        vbc = wf[:, 32:34]
        qkb = wf[0:64, 34:35]
        w2t = wf[0:R, 35:291]
        qkwt = wb[:, 0:CT * 64]
        vwt = wb[:, 128:128 + CT * C]
        g128 = const.tile([P, 1], F32, name="g128")
        nc.gpsimd.dma_start(out=g128, in_=g_d[:, :].to_broadcast([P, 1]))

        ident = const.tile([P, P], BF16, name="ident")
        make_identity(nc, ident)
        ident32 = const.tile([P, P], F32, name="ident32")
        make_identity(nc, ident32)
        # [1, 0] per partition: columns 256/257 of the padded vt tiles
        # (ones -> softmax denominator; zero pad -> keeps the free dim even)
        onespad = const.tile([P, 2], BF16, name="onespad")
        nc.vector.memset(onespad[:, 0:1], 1.0)
        nc.vector.memset(onespad[:, 1:2], 0.0)

        # ---------------- channel attention + bf16 x copy ----------------
        # Pool + cast per DMA chunk so both overlap the x load. The bf16
        # copy is of RAW x: the channel gate is folded into the projection
        # weights instead (q = (Wq diag(g)) x ...), which takes the gate off
        # the critical path of the big casts.
        # PE warmup: the HAM clock gate keeps the PE at half rate until it
        # has seen ~3.4us of sustained matmul activity, and an idle gap
        # >~3.4us re-throttles it. Junk matmuls gated on each arriving x16
        # chunk keep the PE continuously busy through the DMA/pooling
        # prelude so the real matmul stream starts (and stays) at 2.4 GHz.
        x16 = []
        sa4s = []
        s_av = []
        for ct in range(CT):
            xg = xgp.tile([P, HW], BF16, name=f"xg{ct}", tag=f"xg{ct}")
            x16.append(xg)
            sa4s.append(small.tile([P, 2 * NCH], F32, name=f"sa4{ct}",
                                   tag=f"sa4{ct}", bufs=1))
        # chunk-major: process whichever channel-tile's chunk lands first
        for ch in range(NCH):
            for ct in range(CT):
                cw = HW // NCH
                xc = x_sb[ct][:, ch * cw:(ch + 1) * cw]
                # bf16 cast on the (otherwise idle) ACT engine; its fused
                # accumulator produces the sum-pool for free, so the DVE
                # only runs the max-pool
                nc.scalar.activation(out=x16[ct][:, ch * cw:(ch + 1) * cw],
                                     in_=xc, func=AF.Copy,
                                     accum_out=sa4s[ct][:, ch:ch + 1])
                nc.vector.reduce_max(out=sa4s[ct][:, NCH + ch:NCH + ch + 1],
                                     in_=xc, axis=mybir.AxisListType.X)
                # junk matmuls straight on the raw fp32 chunk (4 cyc/row ->
                # long busy time per instruction, no cast dependency)
                jp = ps_t.tile([P, 512], F32, name="jp", tag="tp_ps")
                nc.tensor.matmul(jp, lhsT=ident32, rhs=xc[:, 0:512],
                                 start=True, stop=True)
        for ct in range(CT):
            sa4 = sa4s[ct]
            sa = small.tile([P, 2], F32, name=f"sa{ct}", tag=f"sa{ct}", bufs=1)
            nc.vector.reduce_sum(out=sa[:, 0:1], in_=sa4[:, 0:NCH],
                                 axis=mybir.AxisListType.X)
            nc.vector.reduce_max(out=sa[:, 1:2], in_=sa4[:, NCH:2 * NCH],
                                 axis=mybir.AxisListType.X)
            nc.vector.tensor_scalar_mul(out=sa[:, 0:1], in0=sa[:, 0:1],
                                        scalar1=1.0 / HW)
            s_av.append(sa)

        # bridge junk: gated on the LAST x chunk only, so it fires exactly
        # during the pools-tail + gate computation (the gate matmul below
        # head-blocks the PE queue while waiting for the pools) and keeps
        # the HAM clock warm into the vt/qk phase
        for w in range(6):
            jp = ps_t.tile([P, 512], F32, name="jp", tag="tp_ps")
            nc.tensor.matmul(
                jp, lhsT=ident32,
                rhs=x_sb[w % CT][:, HW - 512:], start=True, stop=True)

        h_ps = ps_o.tile([R, 2], F32, name="h_ps", tag="o_ps")
        for ct in range(CT):
            nc.tensor.matmul(h_ps, lhsT=w1t[:, ct * R:(ct + 1) * R], rhs=s_av[ct],
                             start=(ct == 0), stop=(ct == CT - 1))
        h_sb = small.tile([R, 2], F32, name="h_sb")
        nc.scalar.activation(out=h_sb, in_=h_ps, func=AF.Relu)

        gates = []
        for ct in range(CT):
            g_ps = ps_o.tile([P, 2], F32, name="g_ps", tag="o_ps")
            nc.tensor.matmul(g_ps, lhsT=w2t[:, ct * P:(ct + 1) * P], rhs=h_sb,
                             start=True, stop=True)
            gsb = small.tile([P, 2], F32, name="gsb")
            nc.scalar.activation(out=gsb, in_=g_ps, func=AF.Copy)
            zt = small.tile([P, 1], F32, name="zt")
            nc.vector.tensor_add(out=zt, in0=gsb[:, 0:1], in1=gsb[:, 1:2])
            # sigmoid(z) = 1 / (1 + exp(-z)) -- keeps ACT on the exp table set
            et = small.tile([P, 1], F32, name="et")
            nc.scalar.activation(out=et, in_=zt, func=AF.Exp, scale=-1.0)
            nc.vector.tensor_scalar_add(out=et, in0=et, scalar1=1.0)
            gt = small.tile([P, 1], F32, name=f"gt{ct}", tag=f"gt{ct}", bufs=1)
            nc.vector.reciprocal(out=gt, in_=et)
            gates.append(gt)

        # Fold the gate into the (tiny) projection weights; fold gamma*v_b
        # into the residual (the v bias passes through the softmax average
        # as a constant: V@attn = (Wv xg)@attn + vb, since attn columns sum
        # to 1 after normalization).
        qkwt_g = const.tile([P, CT * 64], BF16, name="qkwt_g")
        vwt_g = const.tile([P, CT * C], BF16, name="vwt_g")
        for ct in range(CT):
            nc.vector.tensor_scalar_mul(
                out=qkwt_g[:, ct * 64:(ct + 1) * 64],
                in0=qkwt[:, ct * 64:(ct + 1) * 64], scalar1=gates[ct])
            # v weights also carry gamma, so the vt PSUM->SBUF move is a
            # plain cast (split ACT/DVE below) instead of a scaled multiply
            gg = small.tile([P, 1], F32, name=f"gg{ct}", tag=f"gg{ct}", bufs=1)
            nc.vector.tensor_mul(out=gg, in0=gates[ct], in1=g128)
            nc.vector.tensor_scalar_mul(
                out=vwt_g[:, ct * C:(ct + 1) * C],
                in0=vwt[:, ct * C:(ct + 1) * C], scalar1=gg)
            gvb = small.tile([P, 1], F32, name=f"gvb{ct}", tag=f"gvb{ct}", bufs=1)
            nc.vector.tensor_mul(out=gvb, in0=vbc[:, ct:ct + 1], in1=g128)
            # x_sb := x*gate + gamma*vb  (residual path, one fused DVE pass)
            nc.vector.tensor_scalar(
                out=x_sb[ct], in0=x_sb[ct], scalar1=gates[ct], scalar2=gvb,
                op0=mybir.AluOpType.mult, op1=mybir.AluOpType.add)

        # ---------------- q, k projections (merged) ----------------
        # One matmul per (i-block, c-tile) computes q AND k: the stationary
        # operand has q_w at output partitions 0-15 and k_w at 32-47
        # (32-aligned partition bases; the zero rows in between are unused).
        # q/k then land in bf16 SBUF tiles replicated at rows 0-15, 32-47,
        # 64-79 and 96-111 so four K=16 score matmuls can run concurrently
        # in PE row-groups 0-3.
        q_sb = qkp.tile([P, HW], BF16, name="q_sb", tag="q_sb")
        k_sb = qkp.tile([P, HW], BF16, name="k_sb", tag="k_sb")
        for ib in range(HW // 512):
            pj = ps_o.tile([64, 512], F32, name="pj", tag="o_ps")
            for ct in range(CT):
                nc.tensor.matmul(
                    pj, lhsT=qkwt_g[:, ct * 64:(ct + 1) * 64],
                    rhs=x16[ct][:, ib * 512:(ib + 1) * 512],
                    start=(ct == 0), stop=(ct == CT - 1))
            # bias-add on DVE (keeps ACT free for the exp stream)
            nc.vector.tensor_scalar_add(
                out=q_sb[0:R, ib * 512:(ib + 1) * 512], in0=pj[0:R, :],
                scalar1=qkb[0:R, :])
            nc.vector.tensor_scalar_add(
                out=k_sb[0:R, ib * 512:(ib + 1) * 512], in0=pj[32:32 + R, :],
                scalar1=qkb[32:32 + R, :])
        for dst in (q_sb, k_sb):
            nc.sync.dma_start(out=dst[32:32 + R, :], in_=dst[0:R, :])

        # ---------------- spatial attention ----------------
        # Engine queues are strict FIFO: an instruction waiting on a
        # semaphore blocks everything emitted after it on the same engine.
        # The emission order below is therefore a software pipeline:
        #   - vt matmuls interleave with scores(0)/exp(0) pair-by-pair, so
        #     the PE fills the ACT exp latency instead of waiting on it
        #   - scores(si+1) pairs interleave into the out2(si) stream
        #     (pt is double-buffered), keeping ACT busy one si ahead
        #   - transposes are deferred one i-tile so they never sit at the
        #     PE queue head waiting for the DVE normalize
        def scores_pair(si, pr, dst):
            # two concurrent K=16 matmuls in PE row-groups 0/1 into one
            # 2-bank PSUM tile; a single exp covers both
            sc_ps = ps_s.tile([P, 2 * SI], F32, name="sc_ps", tag="sc_ps", bufs=2)
            for g in range(2):
                jb = 2 * pr + g
                nc.tensor.matmul(
                    sc_ps[:, g * SI:(g + 1) * SI],
                    lhsT=k_sb[32 * g:32 * g + R, jb * P:(jb + 1) * P],
                    rhs=q_sb[32 * g:32 * g + R, si * SI:(si + 1) * SI],
                    start=True, stop=True,
                    tile_position=(32 * g, 0))
            pt = pp.tile([P, 2 * SI], BF16, name=f"pt{pr}", tag=f"pt{pr}")
            nc.scalar.activation(out=pt, in_=sc_ps, func=AF.Exp)
            dst[pr] = pt

        def junk_mm(n):
            # PE filler during dependency stalls: keeps the HAM activity
            # monitor from re-throttling the clock to 1.2 GHz
            for w in range(n):
                jp = ps_t.tile([P, 512], F32, name="jp", tag="tp_ps")
                nc.tensor.matmul(jp, lhsT=ident, rhs=x16[0][:, 0:512],
                                 start=True, stop=True)

        # vT (gamma*gate folded into vwt_g; ones column for the softmax
        # denominator) interleaved with scores(0)
        p_cur = [None] * (NJ // 2)
        vt_sb = [None] * NJ
        for pr in range(NJ // 2):
            for g in range(2):
                jb = 2 * pr + g
                vt_ps = ps_o.tile([P, C], F32, name="vt_ps", tag="o_ps")
                for ct in range(CT):
                    nc.tensor.matmul(
                        vt_ps, lhsT=x16[ct][:, jb * P:(jb + 1) * P],
                        rhs=vwt_g[:, ct * C:(ct + 1) * C],
                        start=(ct == 0), stop=(ct == CT - 1))
                vt = vtp.tile([P, C + 2], BF16, name=f"vt{jb}", tag=f"vt{jb}")
                nc.vector.tensor_copy(out=vt[:, 0:C], in_=vt_ps)
                nc.vector.tensor_copy(out=vt[:, C:C + 2], in_=onespad)
                vt_sb[jb] = vt
            scores_pair(0, pr, p_cur)

        def p_slice(pairs, jb, lo, hi):
            return pairs[jb // 2][:, (jb % 2) * SI + lo:(jb % 2) * SI + hi]

        pend = []

        def flush_pend():
            while pend:
                r2, i0 = pend.pop(0)
                for ct in range(CT):
                    tp_ps = ps_t.tile([P, P], BF16, name="tp_ps", tag="tp_ps")
                    nc.tensor.transpose(tp_ps, r2[:, ct * P:(ct + 1) * P], ident)
                    y_sb = outp.tile([P, P], F32, name="y_sb", bufs=4)
                    nc.vector.tensor_add(out=y_sb, in0=tp_ps,
                                         in1=x_sb[ct][:, i0:i0 + P])
                    nc.gpsimd.dma_start(
                        out=y_d[ct * P:(ct + 1) * P, i0:i0 + P], in_=y_sb)

        for si in range(NI):
            p_nxt = [None] * (NJ // 2)
            for ii in range(SI // P):
                i0 = si * SI + ii * P
                o_ps = ps_o.tile([P, C + 2], F32, name="o_ps", tag="o_ps")
                for sub in range(4):
                    if si + 1 < NI:
                        scores_pair(si + 1, 4 * ii + sub, p_nxt)
                    for jb in range(8 * sub, 8 * sub + 8):
                        nc.tensor.matmul(
                            o_ps, lhsT=p_slice(p_cur, jb, ii * P, (ii + 1) * P),
                            rhs=vt_sb[jb],
                            start=(jb == 0), stop=(jb == NJ - 1))
                rec = small.tile([P, 1], F32, name="rec")
                nc.vector.reciprocal(out=rec, in_=o_ps[:, C:C + 1])
                r2 = outp.tile([P, C], BF16, name="r2", bufs=3)
                nc.vector.tensor_scalar_mul(out=r2, in0=o_ps[:, 0:C], scalar1=rec)
                flush_pend()
                pend.append((r2, i0))
            p_cur = p_nxt
        flush_pend()
    nc.compile()
    return nc


_NC_CACHE = {}


def _get_nc():
    if "nc" not in _NC_CACHE:
        _NC_CACHE["nc"] = _build_nc()
    return _NC_CACHE["nc"]


def _make_in_maps(inputs):
    x = np.ascontiguousarray(np.asarray(inputs["x"], dtype=np.float32))
    ca_w1 = np.asarray(inputs["ca_w1"], np.float32)
    ca_w2 = np.asarray(inputs["ca_w2"], np.float32)
    q_w = np.asarray(inputs["q_w"], np.float32)
    q_b = np.asarray(inputs["q_b"], np.float32)
    k_w = np.asarray(inputs["k_w"], np.float32)
    k_b = np.asarray(inputs["k_b"], np.float32)
    v_w = np.asarray(inputs["v_w"], np.float32)
    v_b = np.asarray(inputs["v_b"], np.float32)
    gamma = np.asarray(inputs["gamma"], np.float32)

    def ktiles(wT):  # [C, n] -> [128, CT*n] with c-tile-major free dim
        n = wT.shape[1]
        return np.ascontiguousarray(
            wT.reshape(CT, P, n).transpose(1, 0, 2).reshape(P, CT * n))

    bf16 = ml_dtypes.bfloat16
    qk_w = np.zeros((64, C), np.float32)               # q rows 0-15, k rows 32-47
    qk_w[0:R] = q_w
    qk_w[32:32 + R] = k_w
    wf = np.zeros((P, 291), np.float32)
    wf[:, 0:CT * R] = ktiles(ca_w1.T.copy())
    wf[:, 32:34] = v_b.reshape(CT, P).T
    wf[0:R, 34] = q_b
    wf[32:32 + R, 34] = k_b
    wf[0:R, 35:291] = ca_w2.T
    wb = np.zeros((P, 640), np.float32)
    wb[:, 0:CT * 64] = ktiles(qk_w.T.copy())
    wb[:, 128:128 + CT * C] = ktiles(v_w.T.copy())
    shared = {
        "wf": wf,
        "wb": wb.astype(bf16),
        "gamma": gamma.reshape(1, 1).copy(),
    }
    return [{"x": x[b].reshape(C, HW).copy(), **shared} for b in range(B)]


def _run(inputs, trace=False):
    nc = _get_nc()
    in_maps = _make_in_maps(inputs)
    bkr = run_bass_kernel_spmd(nc, in_maps, list(range(B)), trace=trace)
    out = np.stack([np.asarray(bkr.results[b]["y"]).reshape(C, H, W)
                    for b in range(B)])
    return out, bkr


def kernel(**inputs) -> np.ndarray:
    out, _ = _run(inputs, trace=False)
    return out


# revision 55
# speedup vs baseline: 1.2085x; 1.2085x over previous
"""CBAM (channel attention + non-local spatial attention) Trainium2 kernel.

Full-input contract: kernel(**inputs) takes the complete tensors as produced by
setup_inputs() and returns the full [8, 256, 64, 64] output. Internally the
batch dimension (8) is sharded 1:1 across 8 NeuronCores; every core holds the
full (tiny) weights.

Per-core math (batch b, x viewed as [C=256, HW=4096]):
  gate  = sigmoid(W2 relu(W1 avg) + W2 relu(W1 max))          (channel attn)
  xg    = x * gate
  q,k   = Wq/Wk @ xg + b                                      [16, HW]
  vT    = gamma * (xg^T @ Wv^T + v_b)                         [HW, 256]
  S^T   = k^T q                                               [j, i] layout
  P     = exp(S^T)            (scores are O(6); no max-subtraction needed)
  out2  = P^T @ [vT | 1]      -> [i, 256 cols of gamma*V@attn | denom]
  y     = transpose(out2[:, :256] / denom) + xg

All attention-side matmul operands are bf16: the PE streams bf16 at 1
cycle/row (vs 4 for plain fp32) and 128-col bf16 weights get the
compiler's fast-weight-load (2 elems/read), which hides LDWEIGHTS under
the previous matmul's streaming. The channel-attention gate path and the
residual xg stay fp32, so bf16 error only enters through the
gamma~0.05-scaled attention term (~1e-4 end-to-end).
"""

import numpy as np
from contextlib import ExitStack

import ml_dtypes

import concourse.bass as bass
import concourse.tile as tile
from concourse import bacc, mybir
from concourse.bass_utils import run_bass_kernel_spmd
from concourse.masks import make_identity

B, C, H, W = 8, 256, 64, 64
HW = H * W            # 4096
R = 16                # C // reduction
P = 128               # SBUF partitions
CT = C // P           # 2 channel tiles
NJ = HW // P          # 32 key tiles
SI = 512              # query super-block
NI = HW // SI         # 8 super-blocks
F32 = mybir.dt.float32
BF16 = mybir.dt.bfloat16

AF = mybir.ActivationFunctionType


def _build_nc():
    nc = bacc.Bacc("TRN2", target_bir_lowering=False)
    x_d = nc.declare_dram_parameter("x", [C, HW], F32, isOutput=False)
    # all fp32 weights packed in one DRAM blob, all bf16 weights in a
    # second: 2 weight DMAs instead of 6 (fewer DMAs -> no semaphore-pool
    # collision with the x chunk stream)
    # fp32 blob columns: w1t [0:32], vbc [32:34], qkb [34:35] (rows 0-63),
    #                    w2t [35:291] (rows 0-15)
    wf_d = nc.declare_dram_parameter("wf", [P, 291], F32, isOutput=False)
    # bf16 blob columns: qkwt [0:128], vwt [128:640]
    wb_d = nc.declare_dram_parameter("wb", [P, 640], BF16, isOutput=False)
    g_d = nc.declare_dram_parameter("gamma", [1, 1], F32, isOutput=False)
    y_d = nc.declare_dram_parameter("y", [C, HW], F32, isOutput=True)

    with ExitStack() as ctx:
        tc = ctx.enter_context(tile.TileContext(nc))
        const = ctx.enter_context(tc.tile_pool(name="const", bufs=1))
        xp = ctx.enter_context(tc.tile_pool(name="xp", bufs=1))
        xgp = ctx.enter_context(tc.tile_pool(name="xgp", bufs=1))
        qkp = ctx.enter_context(tc.tile_pool(name="qkp", bufs=1))
        vtp = ctx.enter_context(tc.tile_pool(name="vtp", bufs=1))
        # pt double-buffered: exp(si+1) overwrites pt while out2(si) still
        # streams the previous generation -> ACT and PE stay overlapped
        pp = ctx.enter_context(tc.tile_pool(name="pp", bufs=2))
        NQ = NJ // 4          # 8 score quads per si block
        small = ctx.enter_context(tc.tile_pool(name="small", bufs=4))
        outp = ctx.enter_context(tc.tile_pool(name="outp", bufs=4))
        # PSUM: 4 (score quad, single-buffered) + 2 (out2/pj) + 2
        # (transpose/junk) = 8 banks
        ps_s = ctx.enter_context(tc.tile_pool(name="ps_s", bufs=1, space="PSUM"))
        ps_o = ctx.enter_context(tc.tile_pool(name="ps_o", bufs=2, space="PSUM"))
        ps_t = ctx.enter_context(tc.tile_pool(name="ps_t", bufs=2, space="PSUM"))

        # ---------------- loads ----------------
        # A dma_start occupies its issuing engine for the whole transfer,
        # so each data stream gets a dedicated engine: x ct0 on sync, x ct1
        # on gpsimd (idle until the y stores much later), weights on scalar
        # (tiny, emitted ahead of the scalar-engine x16 casts).
        NCH = 4
        x_sb = []
        for ct in range(CT):
            xt = xp.tile([P, HW], F32, name=f"x{ct}", tag=f"x{ct}")
            x_sb.append(xt)
        for ch in range(NCH):
            for ct in range(CT):
                eng = nc.sync if ct == 0 else nc.gpsimd
                eng.dma_start(
                    out=x_sb[ct][:, ch * (HW // NCH):(ch + 1) * (HW // NCH)],
                    in_=x_d[ct * P:(ct + 1) * P,
                            ch * (HW // NCH):(ch + 1) * (HW // NCH)])

        wf = const.tile([P, 291], F32, name="wf")
        nc.scalar.dma_start(out=wf, in_=wf_d[:, :])
        wb = const.tile([P, 640], BF16, name="wb")
        nc.scalar.dma_start(out=wb, in_=wb_d[:, :])
        w1t = wf[:, 0:CT * R]
        vbc = wf[:, 32:34]
        qkb = wf[0:64, 34:35]
        w2t = wf[0:R, 35:291]
        qkwt = wb[:, 0:CT * 64]
        vwt = wb[:, 128:128 + CT * C]
        g128 = const.tile([P, 1], F32, name="g128")
        nc.gpsimd.dma_start(out=g128, in_=g_d[:, :].to_broadcast([P, 1]))

        ident = const.tile([P, P], BF16, name="ident")
        make_identity(nc, ident)
        ident32 = const.tile([P, P], F32, name="ident32")
        make_identity(nc, ident32)
        # [1, 0] per partition: columns 256/257 of the padded vt tiles
        # (ones -> softmax denominator; zero pad -> keeps the free dim even)
        onespad = const.tile([P, 2], BF16, name="onespad")
        nc.vector.memset(onespad[:, 0:1], 1.0)
        nc.vector.memset(onespad[:, 1:2], 0.0)

        # ---------------- channel attention + bf16 x copy ----------------
        # Pool + cast per DMA chunk so both overlap the x load. The bf16
        # copy is of RAW x: the channel gate is folded into the projection
        # weights instead (q = (Wq diag(g)) x ...), which takes the gate off
        # the critical path of the big casts.
        # PE warmup: the HAM clock gate keeps the PE at half rate until it
        # has seen ~3.4us of sustained matmul activity, and an idle gap
        # >~3.4us re-throttles it. Junk matmuls gated on each arriving x16
        # chunk keep the PE continuously busy through the DMA/pooling
        # prelude so the real matmul stream starts (and stays) at 2.4 GHz.
        x16 = []
        sa4s = []
        s_av = []
        for ct in range(CT):
            xg = xgp.tile([P, HW], BF16, name=f"xg{ct}", tag=f"xg{ct}")
            x16.append(xg)
            sa4s.append(small.tile([P, 2 * NCH], F32, name=f"sa4{ct}",
                                   tag=f"sa4{ct}", bufs=1))
        # chunk-major: process whichever channel-tile's chunk lands first
        for ch in range(NCH):
            for ct in range(CT):
                cw = HW // NCH
                xc = x_sb[ct][:, ch * cw:(ch + 1) * cw]
                # bf16 cast on the (otherwise idle) ACT engine; its fused
                # accumulator produces the sum-pool for free, so the DVE
                # only runs the max-pool
                nc.scalar.activation(out=x16[ct][:, ch * cw:(ch + 1) * cw],
                                     in_=xc, func=AF.Copy,
                                     accum_out=sa4s[ct][:, ch:ch + 1])
                nc.vector.reduce_max(out=sa4s[ct][:, NCH + ch:NCH + ch + 1],
                                     in_=xc, axis=mybir.AxisListType.X)
                # junk matmuls straight on the raw fp32 chunk (4 cyc/row ->
                # long busy time per instruction, no cast dependency)
                jp = ps_t.tile([P, 512], F32, name="jp", tag="tp_ps")
                nc.tensor.matmul(jp, lhsT=ident32, rhs=xc[:, 0:512],
                                 start=True, stop=True)
        for ct in range(CT):
            sa4 = sa4s[ct]
            sa = small.tile([P, 2], F32, name=f"sa{ct}", tag=f"sa{ct}", bufs=1)
            nc.vector.reduce_sum(out=sa[:, 0:1], in_=sa4[:, 0:NCH],
                                 axis=mybir.AxisListType.X)
            nc.vector.reduce_max(out=sa[:, 1:2], in_=sa4[:, NCH:2 * NCH],
                                 axis=mybir.AxisListType.X)
            nc.vector.tensor_scalar_mul(out=sa[:, 0:1], in0=sa[:, 0:1],
                                        scalar1=1.0 / HW)
            s_av.append(sa)

        h_ps = ps_o.tile([R, 2], F32, name="h_ps", tag="o_ps")
        for ct in range(CT):
            nc.tensor.matmul(h_ps, lhsT=w1t[:, ct * R:(ct + 1) * R], rhs=s_av[ct],
                             start=(ct == 0), stop=(ct == CT - 1))
        h_sb = small.tile([R, 2], F32, name="h_sb")
        nc.scalar.activation(out=h_sb, in_=h_ps, func=AF.Relu)

        gates = []
        for ct in range(CT):
            g_ps = ps_o.tile([P, 2], F32, name="g_ps", tag="o_ps")
            nc.tensor.matmul(g_ps, lhsT=w2t[:, ct * P:(ct + 1) * P], rhs=h_sb,
                             start=True, stop=True)
            gsb = small.tile([P, 2], F32, name="gsb")
            nc.scalar.activation(out=gsb, in_=g_ps, func=AF.Copy)
            zt = small.tile([P, 1], F32, name="zt")
            nc.vector.tensor_add(out=zt, in0=gsb[:, 0:1], in1=gsb[:, 1:2])
            # sigmoid(z) = 1 / (1 + exp(-z)) -- keeps ACT on the exp table set
            et = small.tile([P, 1], F32, name="et")
            nc.scalar.activation(out=et, in_=zt, func=AF.Exp, scale=-1.0)
            nc.vector.tensor_scalar_add(out=et, in0=et, scalar1=1.0)
            gt = small.tile([P, 1], F32, name=f"gt{ct}", tag=f"gt{ct}", bufs=1)
            nc.vector.reciprocal(out=gt, in_=et)
            gates.append(gt)

        # Fold the gate into the (tiny) projection weights; fold gamma*v_b
        # into the residual (the v bias passes through the softmax average
        # as a constant: V@attn = (Wv xg)@attn + vb, since attn columns sum
        # to 1 after normalization).
        qkwt_g = const.tile([P, CT * 64], BF16, name="qkwt_g")
        vwt_g = const.tile([P, CT * C], BF16, name="vwt_g")
        for ct in range(CT):
            nc.vector.tensor_scalar_mul(
                out=qkwt_g[:, ct * 64:(ct + 1) * 64],
                in0=qkwt[:, ct * 64:(ct + 1) * 64], scalar1=gates[ct])
            # v weights also carry gamma, so the vt PSUM->SBUF move is a
            # plain cast (split ACT/DVE below) instead of a scaled multiply
            gg = small.tile([P, 1], F32, name=f"gg{ct}", tag=f"gg{ct}", bufs=1)
            nc.vector.tensor_mul(out=gg, in0=gates[ct], in1=g128)
            nc.vector.tensor_scalar_mul(
                out=vwt_g[:, ct * C:(ct + 1) * C],
                in0=vwt[:, ct * C:(ct + 1) * C], scalar1=gg)
            gvb = small.tile([P, 1], F32, name=f"gvb{ct}", tag=f"gvb{ct}", bufs=1)
            nc.vector.tensor_mul(out=gvb, in0=vbc[:, ct:ct + 1], in1=g128)
            # x_sb := x*gate + gamma*vb  (residual path, one fused DVE pass)
            nc.vector.tensor_scalar(
                out=x_sb[ct], in0=x_sb[ct], scalar1=gates[ct], scalar2=gvb,
                op0=mybir.AluOpType.mult, op1=mybir.AluOpType.add)

        # ---------------- q, k projections (merged) ----------------
        # One matmul per (i-block, c-tile) computes q AND k: the stationary
        # operand has q_w at output partitions 0-15 and k_w at 32-47
        # (32-aligned partition bases; the zero rows in between are unused).
        # q/k then land in bf16 SBUF tiles replicated at rows 0-15, 32-47,
        # 64-79 and 96-111 so four K=16 score matmuls can run concurrently
        # in PE row-groups 0-3.
        q_sb = qkp.tile([P, HW], BF16, name="q_sb", tag="q_sb")
        k_sb = qkp.tile([P, HW], BF16, name="k_sb", tag="k_sb")
        for ib in range(HW // 512):
            pj = ps_o.tile([64, 512], F32, name="pj", tag="o_ps")
            for ct in range(CT):
                nc.tensor.matmul(
                    pj, lhsT=qkwt_g[:, ct * 64:(ct + 1) * 64],
                    rhs=x16[ct][:, ib * 512:(ib + 1) * 512],
                    start=(ct == 0), stop=(ct == CT - 1))
            # bias-add on DVE (keeps ACT free for the exp stream)
            nc.vector.tensor_scalar_add(
                out=q_sb[0:R, ib * 512:(ib + 1) * 512], in0=pj[0:R, :],
                scalar1=qkb[0:R, :])
            nc.vector.tensor_scalar_add(
                out=k_sb[0:R, ib * 512:(ib + 1) * 512], in0=pj[32:32 + R, :],
                scalar1=qkb[32:32 + R, :])
        for dst in (q_sb, k_sb):
            nc.sync.dma_start(out=dst[32:32 + R, :], in_=dst[0:R, :])

        # ---------------- spatial attention ----------------
        # Engine queues are strict FIFO: an instruction waiting on a
        # semaphore blocks everything emitted after it on the same engine.
        # The emission order below is therefore a software pipeline:
        #   - vt matmuls interleave with scores(0)/exp(0) pair-by-pair, so
        #     the PE fills the ACT exp latency instead of waiting on it
        #   - scores(si+1) pairs interleave into the out2(si) stream
        #     (pt is double-buffered), keeping ACT busy one si ahead
        #   - transposes are deferred one i-tile so they never sit at the
        #     PE queue head waiting for the DVE normalize
        def scores_pair(si, pr, dst):
            # two concurrent K=16 matmuls in PE row-groups 0/1 into one
            # 2-bank PSUM tile; a single exp covers both
            sc_ps = ps_s.tile([P, 2 * SI], F32, name="sc_ps", tag="sc_ps", bufs=2)
            for g in range(2):
                jb = 2 * pr + g
                nc.tensor.matmul(
                    sc_ps[:, g * SI:(g + 1) * SI],
                    lhsT=k_sb[32 * g:32 * g + R, jb * P:(jb + 1) * P],
                    rhs=q_sb[32 * g:32 * g + R, si * SI:(si + 1) * SI],
                    start=True, stop=True,
                    tile_position=(32 * g, 0))
            pt = pp.tile([P, 2 * SI], BF16, name=f"pt{pr}", tag=f"pt{pr}")
            nc.scalar.activation(out=pt, in_=sc_ps, func=AF.Exp)
            dst[pr] = pt

        def junk_mm(n):
            # PE filler during dependency stalls: keeps the HAM activity
            # monitor from re-throttling the clock to 1.2 GHz
            for w in range(n):
                jp = ps_t.tile([P, 512], F32, name="jp", tag="tp_ps")
                nc.tensor.matmul(jp, lhsT=ident, rhs=x16[0][:, 0:512],
                                 start=True, stop=True)

        # vT (gamma*gate folded into vwt_g; ones column for the softmax
        # denominator) interleaved with scores(0)
        p_cur = [None] * (NJ // 2)
        vt_sb = [None] * NJ
        for pr in range(NJ // 2):
            for g in range(2):
                jb = 2 * pr + g
                vt_ps = ps_o.tile([P, C], F32, name="vt_ps", tag="o_ps")
                for ct in range(CT):
                    nc.tensor.matmul(
                        vt_ps, lhsT=x16[ct][:, jb * P:(jb + 1) * P],
                        rhs=vwt_g[:, ct * C:(ct + 1) * C],
                        start=(ct == 0), stop=(ct == CT - 1))
                vt = vtp.tile([P, C + 2], BF16, name=f"vt{jb}", tag=f"vt{jb}")
                nc.vector.tensor_copy(out=vt[:, 0:C], in_=vt_ps)
                nc.vector.tensor_copy(out=vt[:, C:C + 2], in_=onespad)
                vt_sb[jb] = vt
            scores_pair(0, pr, p_cur)

        def p_slice(pairs, jb, lo, hi):
            return pairs[jb // 2][:, (jb % 2) * SI + lo:(jb % 2) * SI + hi]

        pend = []

        def flush_pend():
            while pend:
                r2, i0 = pend.pop(0)
                for ct in range(CT):
                    tp_ps = ps_t.tile([P, P], BF16, name="tp_ps", tag="tp_ps")
                    nc.tensor.transpose(tp_ps, r2[:, ct * P:(ct + 1) * P], ident)
                    y_sb = outp.tile([P, P], F32, name="y_sb", bufs=4)
                    nc.vector.tensor_add(out=y_sb, in0=tp_ps,
                                         in1=x_sb[ct][:, i0:i0 + P])
                    nc.gpsimd.dma_start(
                        out=y_d[ct * P:(ct + 1) * P, i0:i0 + P], in_=y_sb)

        for si in range(NI):
            p_nxt = [None] * (NJ // 2)
            for ii in range(SI // P):
                i0 = si * SI + ii * P
                o_ps = ps_o.tile([P, C + 2], F32, name="o_ps", tag="o_ps")
                for sub in range(4):
                    if si + 1 < NI:
                        scores_pair(si + 1, 4 * ii + sub, p_nxt)
                    for jb in range(8 * sub, 8 * sub + 8):
                        nc.tensor.matmul(
                            o_ps, lhsT=p_slice(p_cur, jb, ii * P, (ii + 1) * P),
                            rhs=vt_sb[jb],
                            start=(jb == 0), stop=(jb == NJ - 1))
                rec = small.tile([P, 1], F32, name="rec")
                nc.vector.reciprocal(out=rec, in_=o_ps[:, C:C + 1])
                r2 = outp.tile([P, C], BF16, name="r2", bufs=3)
                nc.vector.tensor_scalar_mul(out=r2, in0=o_ps[:, 0:C], scalar1=rec)
                flush_pend()
                pend.append((r2, i0))
            p_cur = p_nxt
        flush_pend()
    nc.compile()
    return nc


_NC_CACHE = {}


def _get_nc():
    if "nc" not in _NC_CACHE:
        _NC_CACHE["nc"] = _build_nc()
    return _NC_CACHE["nc"]


def _make_in_maps(inputs):
    x = np.ascontiguousarray(np.asarray(inputs["x"], dtype=np.float32))
    ca_w1 = np.asarray(inputs["ca_w1"], np.float32)
    ca_w2 = np.asarray(inputs["ca_w2"], np.float32)
    q_w = np.asarray(inputs["q_w"], np.float32)
    q_b = np.asarray(inputs["q_b"], np.float32)
    k_w = np.asarray(inputs["k_w"], np.float32)
    k_b = np.asarray(inputs["k_b"], np.float32)
    v_w = np.asarray(inputs["v_w"], np.float32)
    v_b = np.asarray(inputs["v_b"], np.float32)
    gamma = np.asarray(inputs["gamma"], np.float32)

    def ktiles(wT):  # [C, n] -> [128, CT*n] with c-tile-major free dim
        n = wT.shape[1]
        return np.ascontiguousarray(
            wT.reshape(CT, P, n).transpose(1, 0, 2).reshape(P, CT * n))

    bf16 = ml_dtypes.bfloat16
    qk_w = np.zeros((64, C), np.float32)               # q rows 0-15, k rows 32-47
    qk_w[0:R] = q_w
    qk_w[32:32 + R] = k_w
    wf = np.zeros((P, 291), np.float32)
    wf[:, 0:CT * R] = ktiles(ca_w1.T.copy())
    wf[:, 32:34] = v_b.reshape(CT, P).T
    wf[0:R, 34] = q_b
    wf[32:32 + R, 34] = k_b
    wf[0:R, 35:291] = ca_w2.T
    wb = np.zeros((P, 640), np.float32)
    wb[:, 0:CT * 64] = ktiles(qk_w.T.copy())
    wb[:, 128:128 + CT * C] = ktiles(v_w.T.copy())
    shared = {
        "wf": wf,
        "wb": wb.astype(bf16),
        "gamma": gamma.reshape(1, 1).copy(),
    }
    return [{"x": x[b].reshape(C, HW).copy(), **shared} for b in range(B)]


def _run(inputs, trace=False):
    nc = _get_nc()
    in_maps = _make_in_maps(inputs)
    bkr = run_bass_kernel_spmd(nc, in_maps, list(range(B)), trace=trace)
    out = np.stack([np.asarray(bkr.results[b]["y"]).reshape(C, H, W)
                    for b in range(B)])
    return out, bkr


def kernel(**inputs) -> np.ndarray:
    out, _ = _run(inputs, trace=False)
    return out


# revision 59
# speedup vs baseline: 1.2102x; 1.0014x over previous
"""CBAM (channel attention + non-local spatial attention) Trainium2 kernel.

Full-input contract: kernel(**inputs) takes the complete tensors as produced by
setup_inputs() and returns the full [8, 256, 64, 64] output. Internally the
batch dimension (8) is sharded 1:1 across 8 NeuronCores; every core holds the
full (tiny) weights.

Per-core math (batch b, x viewed as [C=256, HW=4096]):
  gate  = sigmoid(W2 relu(W1 avg) + W2 relu(W1 max))          (channel attn)
  xg    = x * gate
  q,k   = Wq/Wk @ xg + b                                      [16, HW]
  vT    = gamma * (xg^T @ Wv^T + v_b)                         [HW, 256]
  S^T   = k^T q                                               [j, i] layout
  P     = exp(S^T)            (scores are O(6); no max-subtraction needed)
  out2  = P^T @ [vT | 1]      -> [i, 256 cols of gamma*V@attn | denom]
  y     = transpose(out2[:, :256] / denom) + xg

All attention-side matmul operands are bf16: the PE streams bf16 at 1
cycle/row (vs 4 for plain fp32) and 128-col bf16 weights get the
compiler's fast-weight-load (2 elems/read), which hides LDWEIGHTS under
the previous matmul's streaming. The channel-attention gate path and the
residual xg stay fp32, so bf16 error only enters through the
gamma~0.05-scaled attention term (~1e-4 end-to-end).
"""

import numpy as np
from contextlib import ExitStack

import ml_dtypes

import concourse.bass as bass
import concourse.tile as tile
from concourse import bacc, mybir
from concourse.bass_utils import run_bass_kernel_spmd
from concourse.masks import make_identity

B, C, H, W = 8, 256, 64, 64
HW = H * W            # 4096
R = 16                # C // reduction
P = 128               # SBUF partitions
CT = C // P           # 2 channel tiles
NJ = HW // P          # 32 key tiles
SI = 512              # query super-block
NI = HW // SI         # 8 super-blocks
F32 = mybir.dt.float32
BF16 = mybir.dt.bfloat16

AF = mybir.ActivationFunctionType


def _build_nc():
    nc = bacc.Bacc("TRN2", target_bir_lowering=False)
    x_d = nc.declare_dram_parameter("x", [C, HW], F32, isOutput=False)
    # all fp32 weights packed in one DRAM blob, all bf16 weights in a
    # second: 2 weight DMAs instead of 6 (fewer DMAs -> no semaphore-pool
    # collision with the x chunk stream)
    # fp32 blob columns: w1t [0:32], vbc [32:34], qkb [34:35] (rows 0-63),
    #                    w2t [35:291] (rows 0-15)
    wf_d = nc.declare_dram_parameter("wf", [P, 291], F32, isOutput=False)
    # bf16 blob columns: qkwt [0:128], vwt [128:640]
    wb_d = nc.declare_dram_parameter("wb", [P, 640], BF16, isOutput=False)
    g_d = nc.declare_dram_parameter("gamma", [1, 1], F32, isOutput=False)
    y_d = nc.declare_dram_parameter("y", [C, HW], F32, isOutput=True)

    with ExitStack() as ctx:
        tc = ctx.enter_context(tile.TileContext(nc))
        const = ctx.enter_context(tc.tile_pool(name="const", bufs=1))
        xp = ctx.enter_context(tc.tile_pool(name="xp", bufs=1))
        xgp = ctx.enter_context(tc.tile_pool(name="xgp", bufs=1))
        qkp = ctx.enter_context(tc.tile_pool(name="qkp", bufs=1))
        vtp = ctx.enter_context(tc.tile_pool(name="vtp", bufs=1))
        # pt double-buffered: exp(si+1) overwrites pt while out2(si) still
        # streams the previous generation -> ACT and PE stay overlapped
        pp = ctx.enter_context(tc.tile_pool(name="pp", bufs=2))
        NQ = NJ // 4          # 8 score quads per si block
        small = ctx.enter_context(tc.tile_pool(name="small", bufs=4))
        outp = ctx.enter_context(tc.tile_pool(name="outp", bufs=4))
        # PSUM: 4 (score quad, single-buffered) + 2 (out2/pj) + 2
        # (transpose/junk) = 8 banks
        ps_s = ctx.enter_context(tc.tile_pool(name="ps_s", bufs=1, space="PSUM"))
        ps_o = ctx.enter_context(tc.tile_pool(name="ps_o", bufs=2, space="PSUM"))
        ps_t = ctx.enter_context(tc.tile_pool(name="ps_t", bufs=2, space="PSUM"))

        # ---------------- loads ----------------
        # A dma_start occupies its issuing engine for the whole transfer,
        # so each data stream gets a dedicated engine: x ct0 on sync, x ct1
        # on gpsimd (idle until the y stores much later), weights on scalar
        # (tiny, emitted ahead of the scalar-engine x16 casts).
        NCH = 4
        x_sb = []
        for ct in range(CT):
            xt = xp.tile([P, HW], F32, name=f"x{ct}", tag=f"x{ct}")
            x_sb.append(xt)
        for ch in range(NCH):
            for ct in range(CT):
                eng = nc.sync if ct == 0 else nc.gpsimd
                eng.dma_start(
                    out=x_sb[ct][:, ch * (HW // NCH):(ch + 1) * (HW // NCH)],
                    in_=x_d[ct * P:(ct + 1) * P,
                            ch * (HW // NCH):(ch + 1) * (HW // NCH)])

        wf = const.tile([P, 291], F32, name="wf")
        nc.scalar.dma_start(out=wf, in_=wf_d[:, :])
        wb = const.tile([P, 640], BF16, name="wb")
        nc.scalar.dma_start(out=wb, in_=wb_d[:, :])
        w1t = wf[:, 0:CT * R]
        vbc = wf[:, 32:34]
        qkb = wf[0:64, 34:35]
        w2t = wf[0:R, 35:291]
        qkwt = wb[:, 0:CT * 64]
        vwt = wb[:, 128:128 + CT * C]
        g128 = const.tile([P, 1], F32, name="g128")
        nc.gpsimd.dma_start(out=g128, in_=g_d[:, :].to_broadcast([P, 1]))

        ident = const.tile([P, P], BF16, name="ident")
        make_identity(nc, ident)
        ident32 = const.tile([P, P], F32, name="ident32")
        make_identity(nc, ident32)
        # [1, 0] per partition: columns 256/257 of the padded vt tiles
        # (ones -> softmax denominator; zero pad -> keeps the free dim even)
        onespad = const.tile([P, 2], BF16, name="onespad")
        nc.vector.memset(onespad[:, 0:1], 1.0)
        nc.vector.memset(onespad[:, 1:2], 0.0)

        # ---------------- channel attention + bf16 x copy ----------------
        # Pool + cast per DMA chunk so both overlap the x load. The bf16
        # copy is of RAW x: the channel gate is folded into the projection
        # weights instead (q = (Wq diag(g)) x ...), which takes the gate off
        # the critical path of the big casts.
        # PE warmup: the HAM clock gate keeps the PE at half rate until it
        # has seen ~3.4us of sustained matmul activity, and an idle gap
        # >~3.4us re-throttles it. Junk matmuls gated on each arriving x16
        # chunk keep the PE continuously busy through the DMA/pooling
        # prelude so the real matmul stream starts (and stays) at 2.4 GHz.
        x16 = []
        sa4s = []
        s_av = []
        for ct in range(CT):
            xg = xgp.tile([P, HW], BF16, name=f"xg{ct}", tag=f"xg{ct}")
            x16.append(xg)
            sa4s.append(small.tile([P, 2 * NCH], F32, name=f"sa4{ct}",
                                   tag=f"sa4{ct}", bufs=1))
        # chunk-major: process whichever channel-tile's chunk lands first
        for ch in range(NCH):
            for ct in range(CT):
                cw = HW // NCH
                xc = x_sb[ct][:, ch * cw:(ch + 1) * cw]
                # bf16 cast on the (otherwise idle) ACT engine; its fused
                # accumulator produces the sum-pool for free, so the DVE
                # only runs the max-pool
                nc.scalar.activation(out=x16[ct][:, ch * cw:(ch + 1) * cw],
                                     in_=xc, func=AF.Copy,
                                     accum_out=sa4s[ct][:, ch:ch + 1])
                nc.vector.reduce_max(out=sa4s[ct][:, NCH + ch:NCH + ch + 1],
                                     in_=xc, axis=mybir.AxisListType.X)
                # junk matmuls straight on the raw fp32 chunk (4 cyc/row ->
                # long busy time per instruction, no cast dependency)
                jp = ps_t.tile([P, 512], F32, name="jp", tag="tp_ps")
                nc.tensor.matmul(jp, lhsT=ident32, rhs=xc[:, 0:512],
                                 start=True, stop=True)
        for ct in range(CT):
            sa4 = sa4s[ct]
            sa = small.tile([P, 2], F32, name=f"sa{ct}", tag=f"sa{ct}", bufs=1)
            nc.vector.reduce_sum(out=sa[:, 0:1], in_=sa4[:, 0:NCH],
                                 axis=mybir.AxisListType.X)
            nc.vector.reduce_max(out=sa[:, 1:2], in_=sa4[:, NCH:2 * NCH],
                                 axis=mybir.AxisListType.X)
            nc.vector.tensor_scalar_mul(out=sa[:, 0:1], in0=sa[:, 0:1],
                                        scalar1=1.0 / HW)
            s_av.append(sa)

        h_ps = ps_o.tile([R, 2], F32, name="h_ps", tag="o_ps")
        for ct in range(CT):
            nc.tensor.matmul(h_ps, lhsT=w1t[:, ct * R:(ct + 1) * R], rhs=s_av[ct],
                             start=(ct == 0), stop=(ct == CT - 1))
        h_sb = small.tile([R, 2], F32, name="h_sb")
        nc.scalar.activation(out=h_sb, in_=h_ps, func=AF.Relu)

        gates = []
        for ct in range(CT):
            g_ps = ps_o.tile([P, 2], F32, name="g_ps", tag="o_ps")
            nc.tensor.matmul(g_ps, lhsT=w2t[:, ct * P:(ct + 1) * P], rhs=h_sb,
                             start=True, stop=True)
            gsb = small.tile([P, 2], F32, name="gsb")
            nc.scalar.activation(out=gsb, in_=g_ps, func=AF.Copy)
            zt = small.tile([P, 1], F32, name="zt")
            nc.vector.tensor_add(out=zt, in0=gsb[:, 0:1], in1=gsb[:, 1:2])
            # sigmoid(z) = 1 / (1 + exp(-z)) -- keeps ACT on the exp table set
            et = small.tile([P, 1], F32, name="et")
            nc.scalar.activation(out=et, in_=zt, func=AF.Exp, scale=-1.0)
            nc.vector.tensor_scalar_add(out=et, in0=et, scalar1=1.0)
            gt = small.tile([P, 1], F32, name=f"gt{ct}", tag=f"gt{ct}", bufs=1)
            nc.vector.reciprocal(out=gt, in_=et)
            gates.append(gt)

        # Fold the gate into the (tiny) projection weights; fold gamma*v_b
        # into the residual (the v bias passes through the softmax average
        # as a constant: V@attn = (Wv xg)@attn + vb, since attn columns sum
        # to 1 after normalization).
        qkwt_g = const.tile([P, CT * 64], BF16, name="qkwt_g")
        vwt_g = const.tile([P, CT * C], BF16, name="vwt_g")
        for ct in range(CT):
            nc.vector.tensor_scalar_mul(
                out=qkwt_g[:, ct * 64:(ct + 1) * 64],
                in0=qkwt[:, ct * 64:(ct + 1) * 64], scalar1=gates[ct])
            # v weights also carry gamma, so the vt PSUM->SBUF move is a
            # plain cast (split ACT/DVE below) instead of a scaled multiply
            gg = small.tile([P, 1], F32, name=f"gg{ct}", tag=f"gg{ct}", bufs=1)
            nc.vector.tensor_mul(out=gg, in0=gates[ct], in1=g128)
            nc.vector.tensor_scalar_mul(
                out=vwt_g[:, ct * C:(ct + 1) * C],
                in0=vwt[:, ct * C:(ct + 1) * C], scalar1=gg)
            gvb = small.tile([P, 1], F32, name=f"gvb{ct}", tag=f"gvb{ct}", bufs=1)
            nc.vector.tensor_mul(out=gvb, in0=vbc[:, ct:ct + 1], in1=g128)
            # x_sb := x*gate + gamma*vb  (residual path, one fused DVE pass)
            nc.vector.tensor_scalar(
                out=x_sb[ct], in0=x_sb[ct], scalar1=gates[ct], scalar2=gvb,
                op0=mybir.AluOpType.mult, op1=mybir.AluOpType.add)

        # ---------------- q, k projections (merged) ----------------
        # One matmul per (i-block, c-tile) computes q AND k: the stationary
        # operand has q_w at output partitions 0-15 and k_w at 32-47
        # (32-aligned partition bases; the zero rows in between are unused).
        # q/k then land in bf16 SBUF tiles replicated at rows 0-15, 32-47,
        # 64-79 and 96-111 so four K=16 score matmuls can run concurrently
        # in PE row-groups 0-3.
        q_sb = qkp.tile([P, HW], BF16, name="q_sb", tag="q_sb")
        k_sb = qkp.tile([P, HW], BF16, name="k_sb", tag="k_sb")
        for ib in range(HW // 512):
            pj = ps_o.tile([64, 512], F32, name="pj", tag="o_ps")
            for ct in range(CT):
                nc.tensor.matmul(
                    pj, lhsT=qkwt_g[:, ct * 64:(ct + 1) * 64],
                    rhs=x16[ct][:, ib * 512:(ib + 1) * 512],
                    start=(ct == 0), stop=(ct == CT - 1))
            # bias-add on DVE (keeps ACT free for the exp stream)
            nc.vector.tensor_scalar_add(
                out=q_sb[0:R, ib * 512:(ib + 1) * 512], in0=pj[0:R, :],
                scalar1=qkb[0:R, :])
            nc.vector.tensor_scalar_add(
                out=k_sb[0:R, ib * 512:(ib + 1) * 512], in0=pj[32:32 + R, :],
                scalar1=qkb[32:32 + R, :])
        for dst in (q_sb, k_sb):
            nc.sync.dma_start(out=dst[32:32 + R, :], in_=dst[0:R, :])

        # ---------------- spatial attention ----------------
        # Engine queues are strict FIFO: an instruction waiting on a
        # semaphore blocks everything emitted after it on the same engine.
        # The emission order below is therefore a software pipeline:
        #   - vt matmuls interleave with scores(0)/exp(0) pair-by-pair, so
        #     the PE fills the ACT exp latency instead of waiting on it
        #   - scores(si+1) pairs interleave into the out2(si) stream
        #     (pt is double-buffered), keeping ACT busy one si ahead
        #   - transposes are deferred one i-tile so they never sit at the
        #     PE queue head waiting for the DVE normalize
        # Query blocks of variable width: the serial exp(block0)+exp(block1)
        # pipeline-fill chain scales with block width, so the first blocks
        # are narrow (the extra per-instruction ACT overhead of the narrow
        # exps hides under the PE-bound steady state).
        BLOCKS = [(512 * i, 512) for i in range(8)]
        NB = len(BLOCKS)

        def scores_pair(bi, pr, dst):
            # two concurrent K=16 matmuls in PE row-groups 0/1 into one
            # 2-bank PSUM tile; a single exp covers both
            off, w = BLOCKS[bi]
            sc_ps = ps_s.tile([P, 2 * SI], F32, name="sc_ps", tag="sc_ps", bufs=2)
            for g in range(2):
                jb = 2 * pr + g
                nc.tensor.matmul(
                    sc_ps[:, g * w:(g + 1) * w],
                    lhsT=k_sb[32 * g:32 * g + R, jb * P:(jb + 1) * P],
                    rhs=q_sb[32 * g:32 * g + R, off:off + w],
                    start=True, stop=True,
                    tile_position=(32 * g, 0))
            pt = pp.tile([P, 2 * SI], BF16, name=f"pt{pr}", tag=f"pt{pr}")
            nc.scalar.activation(out=pt[:, 0:2 * w], in_=sc_ps[:, 0:2 * w],
                                 func=AF.Exp)
            dst[pr] = pt

        def junk_mm(n):
            # PE filler during dependency stalls: keeps the HAM activity
            # monitor from re-throttling the clock to 1.2 GHz
            for w in range(n):
                jp = ps_t.tile([P, 512], F32, name="jp", tag="tp_ps")
                nc.tensor.matmul(jp, lhsT=ident, rhs=x16[0][:, 0:512],
                                 start=True, stop=True)

        # vT (gamma*gate folded into vwt_g; ones column for the softmax
        # denominator) interleaved with scores(0)
        p_cur = [None] * (NJ // 2)
        vt_sb = [None] * NJ
        for pr in range(NJ // 2):
            for g in range(2):
                jb = 2 * pr + g
                vt_ps = ps_o.tile([P, C], F32, name="vt_ps", tag="o_ps")
                for ct in range(CT):
                    nc.tensor.matmul(
                        vt_ps, lhsT=x16[ct][:, jb * P:(jb + 1) * P],
                        rhs=vwt_g[:, ct * C:(ct + 1) * C],
                        start=(ct == 0), stop=(ct == CT - 1))
                vt = vtp.tile([P, C + 2], BF16, name=f"vt{jb}", tag=f"vt{jb}")
                nc.vector.tensor_copy(out=vt[:, 0:C], in_=vt_ps)
                nc.vector.tensor_copy(out=vt[:, C:C + 2], in_=onespad)
                vt_sb[jb] = vt
            scores_pair(0, pr, p_cur)

        def p_slice(pairs, jb, w, lo, hi):
            return pairs[jb // 2][:, (jb % 2) * w + lo:(jb % 2) * w + hi]

        pend = []

        def flush_pend():
            while pend:
                r2, i0 = pend.pop(0)
                for ct in range(CT):
                    tp_ps = ps_t.tile([P, P], BF16, name="tp_ps", tag="tp_ps")
                    nc.tensor.transpose(tp_ps, r2[:, ct * P:(ct + 1) * P], ident)
                    y_sb = outp.tile([P, P], F32, name="y_sb", bufs=4)
                    nc.vector.tensor_add(out=y_sb, in0=tp_ps,
                                         in1=x_sb[ct][:, i0:i0 + P])
                    nc.gpsimd.dma_start(
                        out=y_d[ct * P:(ct + 1) * P, i0:i0 + P], in_=y_sb)

        for bi in range(NB):
            off, w = BLOCKS[bi]
            p_nxt = [None] * (NJ // 2)
            nxt_pairs = NJ // 2 if bi + 1 < NB else 0
            pos_total = (w // P) * 4
            emitted = 0
            for ii in range(w // P):
                i0 = off + ii * P
                o_ps = ps_o.tile([P, C + 2], F32, name="o_ps", tag="o_ps")
                for sub in range(4):
                    want = (4 * ii + sub + 1) * nxt_pairs // pos_total
                    while emitted < want:
                        scores_pair(bi + 1, emitted, p_nxt)
                        emitted += 1
                    for jb in range(8 * sub, 8 * sub + 8):
                        nc.tensor.matmul(
                            o_ps,
                            lhsT=p_slice(p_cur, jb, w, ii * P, (ii + 1) * P),
                            rhs=vt_sb[jb],
                            start=(jb == 0), stop=(jb == NJ - 1))
                rec = small.tile([P, 1], F32, name="rec")
                nc.vector.reciprocal(out=rec, in_=o_ps[:, C:C + 1])
                r2 = outp.tile([P, C], BF16, name="r2", bufs=3)
                nc.vector.tensor_scalar_mul(out=r2, in0=o_ps[:, 0:C], scalar1=rec)
                flush_pend()
                pend.append((r2, i0))
            p_cur = p_nxt
        flush_pend()
    nc.compile()
    return nc


_NC_CACHE = {}


def _get_nc():
    if "nc" not in _NC_CACHE:
        _NC_CACHE["nc"] = _build_nc()
    return _NC_CACHE["nc"]


def _make_in_maps(inputs):
    x = np.ascontiguousarray(np.asarray(inputs["x"], dtype=np.float32))
    ca_w1 = np.asarray(inputs["ca_w1"], np.float32)
    ca_w2 = np.asarray(inputs["ca_w2"], np.float32)
    q_w = np.asarray(inputs["q_w"], np.float32)
    q_b = np.asarray(inputs["q_b"], np.float32)
    k_w = np.asarray(inputs["k_w"], np.float32)
    k_b = np.asarray(inputs["k_b"], np.float32)
    v_w = np.asarray(inputs["v_w"], np.float32)
    v_b = np.asarray(inputs["v_b"], np.float32)
    gamma = np.asarray(inputs["gamma"], np.float32)

    def ktiles(wT):  # [C, n] -> [128, CT*n] with c-tile-major free dim
        n = wT.shape[1]
        return np.ascontiguousarray(
            wT.reshape(CT, P, n).transpose(1, 0, 2).reshape(P, CT * n))

    bf16 = ml_dtypes.bfloat16
    qk_w = np.zeros((64, C), np.float32)               # q rows 0-15, k rows 32-47
    qk_w[0:R] = q_w
    qk_w[32:32 + R] = k_w
    wf = np.zeros((P, 291), np.float32)
    wf[:, 0:CT * R] = ktiles(ca_w1.T.copy())
    wf[:, 32:34] = v_b.reshape(CT, P).T
    wf[0:R, 34] = q_b
    wf[32:32 + R, 34] = k_b
    wf[0:R, 35:291] = ca_w2.T
    wb = np.zeros((P, 640), np.float32)
    wb[:, 0:CT * 64] = ktiles(qk_w.T.copy())
    wb[:, 128:128 + CT * C] = ktiles(v_w.T.copy())
    shared = {
        "wf": wf,
        "wb": wb.astype(bf16),
        "gamma": gamma.reshape(1, 1).copy(),
    }
    return [{"x": x[b].reshape(C, HW).copy(), **shared} for b in range(B)]


def _run(inputs, trace=False):
    nc = _get_nc()
    in_maps = _make_in_maps(inputs)
    bkr = run_bass_kernel_spmd(nc, in_maps, list(range(B)), trace=trace)
    out = np.stack([np.asarray(bkr.results[b]["y"]).reshape(C, H, W)
                    for b in range(B)])
    return out, bkr


def kernel(**inputs) -> np.ndarray:
    out, _ = _run(inputs, trace=False)
    return out
